# revision 4
# baseline (speedup 1.0000x reference)
"""Trainium2 Bass kernel for nn_DiffusionBlock: 20 steps of a 5-point
reflect-padded diffusion stencil on (16, 1, 1024, 1024) fp32.

Approach: the step operator is linear and separable, X <- a*X + Av @ X + X @ Aw^T,
where Av/Aw are 1D tridiagonal-with-reflect-BC operators. Both are
diagonalizable via a diagonal similarity (D = diag(1/2, 1, ..., 1, 1/2)):
Av = D^-1/2 V L V^T D^1/2. The full T-step result is computed spectrally:

    Y = F [ M * (E^T X E) ] F^T,   E = D^1/2 V, F = D^-1/2 V,
    M_ij = (a + lv_i + lw_j)^T  (elementwise mask)

i.e. 4 dense 1024^3 matmuls + 1 elementwise multiply per image — executed on
the TensorEngine in fp32r. Data-parallel over the batch: 2 images per core on
8 NeuronCores, everything SBUF-resident per image.
"""

import os
import sys
import tempfile

import numpy as np

if "/opt/trn_rl_repo" not in sys.path:
    sys.path.insert(0, "/opt/trn_rl_repo")

import concourse.bass as bass  # noqa: E402
import concourse.tile as tile  # noqa: E402
from concourse import bacc, mybir  # noqa: E402
from concourse.bass_utils import run_bass_kernel_spmd  # noqa: E402

N = 1024          # image side
P = 128           # SBUF partitions
NCHUNK = N // P   # 8 chunks per image
F = N * NCHUNK    # flattened free dim of a chunked image tile (8192)
NCORES = 8
IMGS_PER_CORE = 2

_BASS_CACHE = {}
_MAT_CACHE = {}


def _chunk(img):
    """(1024, 1024) -> (128, 8192): chunk k holds rows [128k, 128k+128)."""
    return np.ascontiguousarray(
        img.reshape(NCHUNK, P, N).transpose(1, 0, 2).reshape(P, F)
    )


def _unchunk(t):
    """(128, 8192) -> (1024, 1024)."""
    return np.ascontiguousarray(
        t.reshape(P, NCHUNK, N).transpose(1, 0, 2).reshape(N, N)
    )


def _make_1d_op(n, wm, wp):
    """(A x)[i] = wm*x[i-1] + wp*x[i+1] with jnp.pad(mode='reflect') BC."""
    A = np.zeros((n, n), dtype=np.float64)
    for i in range(n):
        im = 1 if i == 0 else i - 1
        ip = n - 2 if i == n - 1 else i + 1
        A[i, im] += wm
        A[i, ip] += wp
    return A


def _eig_sym(A):
    n = A.shape[0]
    d = np.ones(n)
    d[0] = 0.5
    d[-1] = 0.5
    dh = np.sqrt(d)
    B = (dh[:, None] * A) / dh[None, :]
    assert np.allclose(B, B.T, atol=1e-10), "1D operator not D-symmetrizable"
    lam, V = np.linalg.eigh(B)
    E = dh[:, None] * V      # Z = E^T X E
    Fm = V / dh[:, None]     # Y = F [M*Z] F^T
    return lam, E, Fm


def _build_matrices(weight, time_steps):
    key = (weight.tobytes(), int(time_steps))
    if key in _MAT_CACHE:
        return _MAT_CACHE[key]
    w = np.asarray(weight, dtype=np.float64).reshape(3, 3)
    assert max(abs(w[0, 0]), abs(w[0, 2]), abs(w[2, 0]), abs(w[2, 2])) < 1e-12, (
        "corner weights unsupported by the separable spectral kernel"
    )
    a_c = w[1, 1]
    Av = _make_1d_op(N, w[0, 1], w[2, 1])    # vertical (rows)
    Aw = _make_1d_op(N, w[1, 0], w[1, 2])    # horizontal (cols)
    lv, Ev, Fv = _eig_sym(Av)
    lw, Ew, Fw = _eig_sym(Aw)
    mask = (a_c + lv[:, None] + lw[None, :]) ** int(time_steps)
    # identical operators both axes for this problem; keep general anyway
    assert np.allclose(Av, Aw), "asymmetric stencils need separate E/F per axis"
    em = _chunk(Ev).astype(np.float32)           # E, contracted over first index
    ft = _chunk(Fv.T.copy()).astype(np.float32)  # F^T, contracted over first index
    mk = _chunk(mask).astype(np.float32)
    _MAT_CACHE[key] = (em, ft, mk)
    return em, ft, mk


def _stage(nc, psum_pool, in_t, const_t, out_t, mask_t=None, out_f32=False):
    """out = in^T @ const per 128-col tile of the output, PSUM-accumulated
    over the 8 row-chunks. in_t is the stationary operand (chunk layout),
    const_t the moving one. PSUM is drained to out_t (optionally multiplied
    elementwise by mask_t)."""
    for m in range(NCHUNK):
        ps = psum_pool.tile([P, N], mybir.dt.float32, tag="ps")
        for k in range(NCHUNK):
            base = N * k
            lhsT = in_t[:, base + P * m: base + P * (m + 1)]
            for nh in range(2):
                nc.tensor.matmul(
                    out=ps[:, 512 * nh: 512 * (nh + 1)],
                    lhsT=lhsT,
                    rhs=const_t[:, base + 512 * nh: base + 512 * (nh + 1)],
                    start=(k == 0),
                    stop=(k == NCHUNK - 1),
                )
        out_ap = out_t[:, N * m: N * (m + 1)]
        if mask_t is not None:
            nc.vector.tensor_tensor(
                out=out_ap, in0=ps[:, :],
                in1=mask_t[:, N * m: N * (m + 1)],
                op=mybir.AluOpType.mult,
            )
        else:
            nc.vector.tensor_copy(out=out_ap, in_=ps[:, :])


def _build_bass():
    if "nc" in _BASS_CACHE:
        return _BASS_CACHE["nc"]
    nc = bacc.Bacc("TRN2", target_bir_lowering=False, debug=False,
                   num_devices=NCORES)
    f32 = mybir.dt.float32
    f32r = mybir.dt.float32r
    xs_d = nc.dram_tensor("xs", [IMGS_PER_CORE, P, F], f32r, kind="ExternalInput").ap()
    em_d = nc.dram_tensor("em", [P, F], f32r, kind="ExternalInput").ap()
    ft_d = nc.dram_tensor("ft", [P, F], f32r, kind="ExternalInput").ap()
    mk_d = nc.dram_tensor("mk", [P, F], f32, kind="ExternalInput").ap()
    ys_d = nc.dram_tensor("ys", [IMGS_PER_CORE, P, F], f32, kind="ExternalOutput").ap()

    with tile.TileContext(nc) as tc:
        with tc.tile_pool(name="const", bufs=1) as cpool, \
             tc.tile_pool(name="data", bufs=1) as dpool, \
             tc.tile_pool(name="psum", bufs=4, space="PSUM") as ppool:
            em_t = cpool.tile([P, F], f32r, tag="em")
            ft_t = cpool.tile([P, F], f32r, tag="ft")
            mk_t = cpool.tile([P, F], f32, tag="mk")
            nc.sync.dma_start(out=em_t[:, :], in_=em_d[:, :])
            nc.sync.dma_start(out=ft_t[:, :], in_=ft_d[:, :])
            nc.sync.dma_start(out=mk_t[:, :], in_=mk_d[:, :])

            for img in range(IMGS_PER_CORE):
                xa = dpool.tile([P, F], f32r, tag="bufA")
                nc.sync.dma_start(out=xa[:, :], in_=xs_d[img, :, :])

                # S1: W = X^T E  (chunked by c)
                wb = dpool.tile([P, F], f32r, tag="bufB")
                _stage(nc, ppool, xa, em_t, wb)
                # S2+S3: Z = W^T E, G = M * Z  (chunked by j1)
                ga = dpool.tile([P, F], f32r, tag="bufA")
                _stage(nc, ppool, wb, em_t, ga, mask_t=mk_t)
                # S4: H = G^T Ft  (chunked by j2)
                hb = dpool.tile([P, F], f32r, tag="bufB")
                _stage(nc, ppool, ga, ft_t, hb)
                # S5: Y = H^T Ft  (chunked by r)
                yc = dpool.tile([P, F], f32, tag="bufC")
                _stage(nc, ppool, hb, ft_t, yc)
                nc.sync.dma_start(out=ys_d[img, :, :], in_=yc[:, :])

    nc.compile()
    _BASS_CACHE["nc"] = nc
    return nc


def kernel(x, weight, time_steps, **_ignored):
    x = np.asarray(x, dtype=np.float32)
    weight = np.asarray(weight, dtype=np.float32)
    em, ft, mk = _build_matrices(weight, time_steps)
    nc = _build_bass()

    b = x.shape[0]
    assert b == NCORES * IMGS_PER_CORE and x.shape[-2:] == (N, N)
    in_maps = []
    for c in range(NCORES):
        xs = np.stack([
            _chunk(x[c * IMGS_PER_CORE + i, 0]) for i in range(IMGS_PER_CORE)
        ])
        in_maps.append({"xs": xs, "em": em, "ft": ft, "mk": mk})

    res = run_bass_kernel_spmd(nc, in_maps, core_ids=list(range(NCORES)))
    _BASS_CACHE["last_results"] = res

    out = np.empty((b, 1, N, N), dtype=np.float32)
    for c in range(NCORES):
        ys = res.results[c]["ys"]
        for i in range(IMGS_PER_CORE):
            out[c * IMGS_PER_CORE + i, 0] = _unchunk(ys[i])
    return out


# revision 12
# speedup vs baseline: 1.3417x; 1.3417x over previous
"""Trainium2 Bass kernel for nn_DiffusionBlock: 20 steps of a 5-point
reflect-padded diffusion stencil on (16, 1, 1024, 1024) fp32.

Approach: the step operator is linear and separable, X <- a*X + Av @ X + X @ Aw^T,
where Av/Aw are 1D tridiagonal-with-reflect-BC operators. Both are
diagonalizable via a diagonal similarity (D = diag(1/2, 1, ..., 1, 1/2)):
Av = D^-1/2 V L V^T D^1/2. The full T-step result is computed spectrally:

    Y = F [ M * (E^T X E) ] F^T,   E = D^1/2 V, F = D^-1/2 V,
    M_ij = (a + lv_i + lw_j)^T  (elementwise mask)

i.e. 4 dense 1024^3 matmuls + 1 elementwise multiply per image — executed on
the TensorEngine in fp32r. Data-parallel over the batch: 2 images per core on
8 NeuronCores, everything SBUF-resident per image.
"""

import os
import sys
import tempfile

import numpy as np

if "/opt/trn_rl_repo" not in sys.path:
    sys.path.insert(0, "/opt/trn_rl_repo")

import concourse.bass as bass  # noqa: E402
import concourse.tile as tile  # noqa: E402
from concourse import bacc, mybir  # noqa: E402
from concourse.bass_utils import run_bass_kernel_spmd  # noqa: E402

N = 1024          # image side
P = 128           # SBUF partitions
NCHUNK = N // P   # 8 chunks per image
F = N * NCHUNK    # flattened free dim of a chunked image tile (8192)
NCORES = 8
IMGS_PER_CORE = 2

_BASS_CACHE = {}
_MAT_CACHE = {}


def _chunk(img):
    """(1024, 1024) -> (128, 8192): chunk k holds rows [128k, 128k+128)."""
    return np.ascontiguousarray(
        img.reshape(NCHUNK, P, N).transpose(1, 0, 2).reshape(P, F)
    )


def _unchunk(t):
    """(128, 8192) -> (1024, 1024)."""
    return np.ascontiguousarray(
        t.reshape(P, NCHUNK, N).transpose(1, 0, 2).reshape(N, N)
    )


def _make_1d_op(n, wm, wp):
    """(A x)[i] = wm*x[i-1] + wp*x[i+1] with jnp.pad(mode='reflect') BC."""
    A = np.zeros((n, n), dtype=np.float64)
    for i in range(n):
        im = 1 if i == 0 else i - 1
        ip = n - 2 if i == n - 1 else i + 1
        A[i, im] += wm
        A[i, ip] += wp
    return A


def _eig_sym(A):
    n = A.shape[0]
    d = np.ones(n)
    d[0] = 0.5
    d[-1] = 0.5
    dh = np.sqrt(d)
    B = (dh[:, None] * A) / dh[None, :]
    assert np.allclose(B, B.T, atol=1e-10), "1D operator not D-symmetrizable"
    lam, V = np.linalg.eigh(B)
    E = dh[:, None] * V      # Z = E^T X E
    Fm = V / dh[:, None]     # Y = F [M*Z] F^T
    return lam, E, Fm


def _build_matrices(weight, time_steps):
    key = (weight.tobytes(), int(time_steps))
    if key in _MAT_CACHE:
        return _MAT_CACHE[key]
    w = np.asarray(weight, dtype=np.float64).reshape(3, 3)
    assert max(abs(w[0, 0]), abs(w[0, 2]), abs(w[2, 0]), abs(w[2, 2])) < 1e-12, (
        "corner weights unsupported by the separable spectral kernel"
    )
    a_c = w[1, 1]
    Av = _make_1d_op(N, w[0, 1], w[2, 1])    # vertical (rows)
    Aw = _make_1d_op(N, w[1, 0], w[1, 2])    # horizontal (cols)
    lv, Ev, Fv = _eig_sym(Av)
    lw, Ew, Fw = _eig_sym(Aw)
    mask = (a_c + lv[:, None] + lw[None, :]) ** int(time_steps)
    # identical operators both axes for this problem; keep general anyway
    assert np.allclose(Av, Aw), "asymmetric stencils need separate E/F per axis"
    em = _chunk(Ev).astype(np.float32)           # E, contracted over first index
    ft = _chunk(Fv.T.copy()).astype(np.float32)  # F^T, contracted over first index
    mk = _chunk(mask).astype(np.float32)

    # Mask block sparsity: mid-band spectral blocks decay to ~0 after T steps,
    # so the corresponding stage-2 output halves and stage-4 stationary blocks
    # can be skipped entirely (error contribution < 1e-6 at TH=1e-5).
    TH = 1e-5
    zero_half = tuple(
        tuple(bool(np.max(np.abs(mask[P * m: P * (m + 1),
                                      512 * nh: 512 * (nh + 1)])) < TH)
              for nh in range(2))
        for m in range(NCHUNK))
    zero_blk = tuple(
        tuple(bool(np.max(np.abs(mask[P * k: P * (k + 1),
                                      P * m: P * (m + 1)])) < TH)
              for m in range(NCHUNK))
        for k in range(NCHUNK))
    for m in range(NCHUNK):
        assert not all(zero_blk[k][m] for k in range(NCHUNK))
        assert not all(zero_half[m])
    _MAT_CACHE[key] = (em, ft, mk, zero_half, zero_blk)
    return _MAT_CACHE[key]


def _stage(nc, psum_pool, in_t, const_t, out_t, mask_t=None,
           skip_halves=None, skip_blocks=None, evac_engine="scalar"):
    """out = in^T @ const per 128-col tile of the output, PSUM-accumulated
    over the 8 row-chunks. in_t is the stationary operand (chunk layout),
    const_t the moving one. PSUM is drained to out_t (optionally multiplied
    elementwise by mask_t). skip_halves[m][nh] skips whole output halves
    (writes zeros); skip_blocks[k][m] skips zero stationary blocks."""
    for m in range(NCHUNK):
        live_nh = [nh for nh in range(2)
                   if not (skip_halves and skip_halves[m][nh])]
        live_k = [k for k in range(NCHUNK)
                  if not (skip_blocks and skip_blocks[k][m])]
        ps = psum_pool.tile([P, N], mybir.dt.float32, tag="ps")
        for k in live_k:
            base = N * k
            lhsT = in_t[:, base + P * m: base + P * (m + 1)]
            for nh in live_nh:
                nc.tensor.matmul(
                    out=ps[:, 512 * nh: 512 * (nh + 1)],
                    lhsT=lhsT,
                    rhs=const_t[:, base + 512 * nh: base + 512 * (nh + 1)],
                    start=(k == live_k[0]),
                    stop=(k == live_k[-1]),
                )
        # skipped halves are left unwritten: every downstream read of a zero
        # half is itself a skipped stage-4 block (zero_half[m][nh] implies
        # zero_blk for all its sub-blocks), so nothing ever reads them
        spans = ([(0, N)] if len(live_nh) == 2 else
                 [(512 * nh, 512 * (nh + 1)) for nh in live_nh])
        for lo, hi in spans:
            out_ap = out_t[:, N * m + lo: N * m + hi]
            ps_ap = ps[:, lo: hi]
            if mask_t is not None:
                nc.vector.tensor_tensor(
                    out=out_ap, in0=ps_ap,
                    in1=mask_t[:, N * m + lo: N * m + hi],
                    op=mybir.AluOpType.mult,
                )
            elif evac_engine == "scalar":
                nc.scalar.copy(out=out_ap, in_=ps_ap)
            else:
                nc.vector.tensor_copy(out=out_ap, in_=ps_ap)


def _build_bass(zero_half, zero_blk):
    cache_key = (zero_half, zero_blk)
    if cache_key in _BASS_CACHE:
        return _BASS_CACHE[cache_key]
    nc = bacc.Bacc("TRN2", target_bir_lowering=False, debug=False,
                   num_devices=NCORES)
    f32 = mybir.dt.float32
    f32r = mybir.dt.float32r
    xs_d = nc.dram_tensor("xs", [IMGS_PER_CORE, P, F], f32r, kind="ExternalInput").ap()
    em_d = nc.dram_tensor("em", [P, F], f32r, kind="ExternalInput").ap()
    ft_d = nc.dram_tensor("ft", [P, F], f32r, kind="ExternalInput").ap()
    mk_d = nc.dram_tensor("mk", [P, F], f32, kind="ExternalInput").ap()
    ys_d = nc.dram_tensor("ys", [IMGS_PER_CORE, P, F], f32, kind="ExternalOutput").ap()

    with tile.TileContext(nc) as tc:
        with tc.tile_pool(name="const", bufs=1) as cpool, \
             tc.tile_pool(name="data", bufs=1) as dpool, \
             tc.tile_pool(name="psum", bufs=4, space="PSUM") as ppool:
            em_t = cpool.tile([P, F], f32r, tag="em")
            ft_t = cpool.tile([P, F], f32r, tag="ft")
            mk_t = cpool.tile([P, F], f32, tag="mk")
            # per-chunk DMAs so stage-1 can start on chunk 0 immediately
            for k in range(NCHUNK):
                s = slice(N * k, N * (k + 1))
                nc.sync.dma_start(out=em_t[:, s], in_=em_d[:, s])
            for k in range(NCHUNK):
                s = slice(N * k, N * (k + 1))
                nc.sync.dma_start(out=ft_t[:, s], in_=ft_d[:, s])
            for k in range(NCHUNK):
                s = slice(N * k, N * (k + 1))
                nc.sync.dma_start(out=mk_t[:, s], in_=mk_d[:, s])

            for img in range(IMGS_PER_CORE):
                xa = dpool.tile([P, F], f32r, tag="bufA")
                for k in range(NCHUNK):
                    s = slice(N * k, N * (k + 1))
                    nc.sync.dma_start(out=xa[:, s], in_=xs_d[img, :, s])

                # S1: W = X^T E  (chunked by c)
                wb = dpool.tile([P, F], f32r, tag="bufB")
                _stage(nc, ppool, xa, em_t, wb)
                # S2+S3: Z = W^T E, G = M * Z  (chunked by j1)
                ga = dpool.tile([P, F], f32r, tag="bufA")
                _stage(nc, ppool, wb, em_t, ga, mask_t=mk_t,
                       skip_halves=zero_half)
                # S4: H = G^T Ft  (chunked by j2)
                hb = dpool.tile([P, F], f32r, tag="bufB")
                _stage(nc, ppool, ga, ft_t, hb, skip_blocks=zero_blk)
                # S5: Y = H^T Ft  (chunked by r)
                yc = dpool.tile([P, F], f32, tag="bufC")
                _stage(nc, ppool, hb, ft_t, yc)
                nc.sync.dma_start(out=ys_d[img, :, :], in_=yc[:, :])

    nc.compile()
    _BASS_CACHE[cache_key] = nc
    return nc


def kernel(x, weight, time_steps, **_ignored):
    x = np.asarray(x, dtype=np.float32)
    weight = np.asarray(weight, dtype=np.float32)
    em, ft, mk, zero_half, zero_blk = _build_matrices(weight, time_steps)
    nc = _build_bass(zero_half, zero_blk)

    b = x.shape[0]
    assert b == NCORES * IMGS_PER_CORE and x.shape[-2:] == (N, N)
    in_maps = []
    for c in range(NCORES):
        xs = np.stack([
            _chunk(x[c * IMGS_PER_CORE + i, 0]) for i in range(IMGS_PER_CORE)
        ])
        in_maps.append({"xs": xs, "em": em, "ft": ft, "mk": mk})

    res = run_bass_kernel_spmd(nc, in_maps, core_ids=list(range(NCORES)))
    _BASS_CACHE["last_results"] = res

    out = np.empty((b, 1, N, N), dtype=np.float32)
    for c in range(NCORES):
        ys = res.results[c]["ys"]
        for i in range(IMGS_PER_CORE):
            out[c * IMGS_PER_CORE + i, 0] = _unchunk(ys[i])
    return out
